# revision 14
# baseline (speedup 1.0000x reference)
"""Single-head attention (B=8, S=2048, E=768, D=64) on 8 TRN2 NeuronCores.

Sharding: data-parallel over batch — one batch element per core; the small
Wq/Wk/Wv weights and biases are replicated to every core.

Per-core dataflow (fp16 matmul path, fp32 PSUM accumulation; rel err ~9e-4):

  Setup: exp-table load first on ACT; H s-tile loads stream on the SWDGE
  (gpsimd) queue with inline f32->fp16 cast (tile 0 split in 3 pieces so the
  PE can start sooner); the weight loads are SWDGE cast-loads slotted after
  the first 6 H tiles; the 1/sqrt(D) softmax scale is folded into the exp
  ACTIVATE's free affine (out = exp(scale*x)) so no weight prep is needed;
  8 fp16 warm-up matmuls keep the PE HAM activity window busy so the clock
  gate reaches K=8/8 before real work.

  Phase A (per 512-query chunk): PE-transpose H tiles against a fp16
  identity (normal matmuls, full-array activity), evacuate PSUM->SBUF on
  DVE (chunk 0 on ACT — it is otherwise idle until the first scores exist),
  project qkT = [Wq | Wk].T @ HT and vT = Wv.T @ HT with biases folded into
  the evacuation, copy kT down to partitions 0:64 (kT_lo) and qT up to
  partitions 64:128 (qT_hi) via SBUF->SBUF DMA on the HWDGE queue (which
  carries only the tiny bias loads besides), and PE-transpose v into
  per-key-tile [128,128] tiles (col 64 = 1.0: softmax-denominator trick).

  Attention: scores per (query-chunk c, key-pair p) tile are TWO CONCURRENT
  K=64 matmuls packed into PE row groups 0/64 (tile_position packing:
  kT_lo x qT in rows 0:64, qkT-hi x qT_hi in rows 64:128) -> one
  [128, 1024] PSUM tile; exp on ACT -> persistent SBUF fp16.  QK^T+exp for
  ready tiles are emitted INSIDE phase A (early attention) so ACT's serial
  exp stream — the largest single-engine cost, ~(1024+352)/1.2 ns x 32 —
  starts as soon as the first chunk is projected.  PV is deferred to
  phase B: per 1024-query group it accumulates v_sb.T @ expT over all 16
  key tiles (ones row = denominator), with the remaining QK^T+exp tiles
  fed into the stream at a rate of one per PV tile so the ACT pipeline
  never starves and PSUM score slots never block the in-order PE queue.

  Epilogue per query group: evacuate PV PSUM->SBUF as fp16 (DVE),
  PE-transpose back to [queries, 65], one strided reciprocal over all 8
  denominator columns, 8 tensor_scalar multiplies, store via HWDGE.

Softmax without max-subtraction is safe here: scores/8 ~ N(0,1) (max
|score/8| < ~8 over the whole problem), so exp() <= ~3000 and the fp16
denominators stay far below overflow; the result matches the
max-subtracted reference to fp32 rounding.
"""

from collections import deque
from contextlib import ExitStack

import numpy as np

import concourse.bacc as bacc
import concourse.mybir as mybir
import concourse.tile as tile
from concourse.bass_utils import run_bass_kernel_spmd

B = 8
S = 2048
E = 768
D = 64
P = 128
NT_S = S // P  # 16 key/s-tiles
NT_E = E // P  # 6 e-tiles
CH = 512  # query-chunk width
NCH = S // CH  # 4 query chunks
NPAIR = NT_S // 2  # 8 key-tile pairs
F32 = mybir.dt.float32
F16 = mybir.dt.float16
AF = mybir.ActivationFunctionType

SCALE = float(1.0 / np.sqrt(np.float32(D)))

# (chunk, pair) attention tiles emitted inside phase A, keyed by (chunk,
# s-tile-slot) DURING whose processing they are emitted.  Tile (c, p) needs
# qkT chunk c and kT tiles 2p,2p+1 (chunk p//2) plus the kT_lo/qT_hi copies
# of those chunks, so during chunk k anything with c <= k-1, p <= 2k-1 is
# ready (slot >= 1 leaves ~1 s-tile of slack for the copy DMA latency).
EARLY_SCHED = {
    (1, 1): [(0, 0)],
    (1, 2): [(0, 1)],
    (2, 1): [(1, 0)],
    (2, 2): [(1, 1)],
    (2, 3): [(0, 2), (0, 3)],
    (3, 0): [(1, 2)],
    (3, 1): [(1, 3), (2, 0)],
    (3, 2): [(2, 1), (2, 2)],
    (3, 3): [(2, 3)],
}
EARLY = [t for lst in EARLY_SCHED.values() for t in lst]
# Remaining tiles, in phase-B emission order, co-designed with the PV
# consumption order (late pairs p=4..7 first, early pairs p=0..3 last) so
# every group's PV stream ENDS on tiles whose exp finished long before.
LATE = [
    (0, 4), (1, 4), (0, 5), (1, 5), (0, 6), (1, 6), (0, 7), (1, 7),
    (2, 4), (3, 4), (2, 5), (3, 5), (2, 6), (3, 6), (2, 7), (3, 7),
    (3, 0), (3, 1), (3, 2), (3, 3),
]
PV_PSEQ = [4, 5, 6, 7, 0, 1, 2, 3]


def _emit_kernel(ctx: ExitStack, tc: "tile.TileContext", o, h, wq, bq, wk, bk, wv, bv):
    nc = tc.nc

    const = ctx.enter_context(tc.tile_pool(name="const", bufs=1))
    hload = ctx.enter_context(tc.tile_pool(name="hload", bufs=1))
    htp = ctx.enter_context(tc.tile_pool(name="htp", bufs=2))
    big = ctx.enter_context(tc.tile_pool(name="bigsb", bufs=1))
    outp = ctx.enter_context(tc.tile_pool(name="outp", bufs=2))

    # --- setup ------------------------------------------------------------
    # Dummy exp first so the ACT exp table set loads during the DMA ramp.
    dummy = const.tile([1, 4], F32)
    nc.vector.memset(dummy[:], 0.0)
    nc.scalar.activation(dummy[:], dummy[:], AF.Exp)

    # H tile loads: SWDGE casts f32->fp16 inline.  Tile 0 is split in three
    # so its first third lands earlier and transposes start sooner.  The
    # weight cast-loads are slotted after the first 6 H tiles: early enough
    # for the first projection, without delaying h0.
    h_tiles = [hload.tile([P, E], F16, name=f"h{st}") for st in range(NT_S)]
    TE2 = 2 * P
    for piece in range(3):
        nc.gpsimd.dma_start(
            h_tiles[0][:, piece * TE2 : (piece + 1) * TE2],
            h[0:P, piece * TE2 : (piece + 1) * TE2],
        )
    nc.gpsimd.dma_start(h_tiles[1][:], h[P : 2 * P, :])

    # identity: DVE zeros + gpsimd diagonal (after h0/h1 descriptors so it
    # does not delay them; ready by the first transpose).
    ident = const.tile([P, P], F32)
    nc.vector.memset(ident[:], 0.0)
    nc.gpsimd.affine_select(
        out=ident[:],
        in_=ident[:],
        compare_op=mybir.AluOpType.not_equal,
        fill=1.0,
        base=0,
        pattern=[[-1, P]],
        channel_multiplier=1,
    )
    ident_h = const.tile([P, P], F16)
    nc.vector.tensor_copy(ident_h[:], ident[:])

    for st in range(2, NT_S):
        nc.gpsimd.dma_start(h_tiles[st][:], h[st * P : (st + 1) * P, :])

    # weights: f32 on the HWDGE sync queue (parallel to the H stream on the
    # SWDGE queue), cast to fp16 on the then-idle DVE
    wq_raw = const.tile([P, NT_E, D], F32)
    wk_raw = const.tile([P, NT_E, D], F32)
    wv_raw = const.tile([P, NT_E, D], F32)
    nc.sync.dma_start(wq_raw[:], wq.rearrange("(t p) d -> p t d", p=P))
    nc.sync.dma_start(wk_raw[:], wk.rearrange("(t p) d -> p t d", p=P))
    nc.sync.dma_start(wv_raw[:], wv.rearrange("(t p) d -> p t d", p=P))
    wqk = const.tile([P, NT_E, P], F16)  # cols 0:64 = Wq, 64:128 = Wk
    wv_h = const.tile([P, NT_E, D], F16)
    nc.vector.tensor_copy(wqk[:, :, 0:D], wq_raw[:])
    nc.vector.tensor_copy(wqk[:, :, D:P], wk_raw[:])
    nc.vector.tensor_copy(wv_h[:], wv_raw[:])

    # Warm-up: fp16 matmuls (~430ns each cold) keep the PE busy through the
    # HAM activity window while the first H tiles stream in.
    warm = const.tile([P, CH], F16)
    nc.vector.memset(warm[:], 1.0)
    with tc.tile_pool(name="ps_warm", bufs=1, space="PSUM") as ps_warm:
        warm_ps = ps_warm.tile([P, CH], F32)
        for _ in range(8):
            nc.tensor.matmul(warm_ps[:], warm[:, 0:P], warm[:], start=True, stop=True)

    # biases: tiny f32 loads on the otherwise-idle HWDGE sync queue
    bias_qk = const.tile([P, 1], F32)
    nc.sync.dma_start(bias_qk[0:D, :], bq.rearrange("(p one) -> p one", one=1))
    nc.sync.dma_start(bias_qk[D:P, :], bk.rearrange("(p one) -> p one", one=1))
    bias_v = const.tile([D, 1], F32)
    nc.sync.dma_start(bias_v[:], bv.rearrange("(p one) -> p one", one=1))

    # --- persistent SBUF --------------------------------------------------
    qkT = big.tile([P, S], F16)  # rows 0:64 = qT, rows 64:128 = kT
    qT_hi = big.tile([P, S], F16)  # rows 64:128 = qT (for row-group-64 QK^T)
    kT_lo = big.tile([D, S], F16)  # partitions 0:64 = kT (row-group-0 QK^T)
    vT = big.tile([D, S], F16)
    v_sb = big.tile([P, NT_S, P], F16)  # per key tile: [s, 0:64]=v, col64=1, rest 0
    nc.vector.memset(v_sb[:], 0.0)
    nc.vector.memset(v_sb[:, :, D : D + 1], 1.0)
    expT = big.tile([P, NCH * NPAIR, 2 * CH], F16)  # exp tile per (c, p)
    o_acc = big.tile([P, NT_S * D], F32)

    emitted = set()

    def emit_qkt_exp(pool, c, p):
        """Row-packed QK^T pair + exp (with the 1/8 scale folded in)."""
        s_ps = pool.tile([P, 2 * CH], F32, tag="s")
        nc.tensor.matmul(
            s_ps[:, 0:CH],
            kT_lo[:, 2 * p * P : (2 * p + 1) * P],
            qkT[0:D, c * CH : (c + 1) * CH],
            start=True,
            stop=True,
        )
        nc.tensor.matmul(
            s_ps[:, CH : 2 * CH],
            qkT[D:P, (2 * p + 1) * P : (2 * p + 2) * P],
            qT_hi[D:P, c * CH : (c + 1) * CH],
            start=True,
            stop=True,
        )
        nc.scalar.activation(expT[:, c * NPAIR + p, :], s_ps[:], AF.Exp, scale=SCALE)
        emitted.add((c, p))

    # --- phase A: transpose H, project, early attention -------------------
    # PSUM: ps_sA 1x4KB + ps_ht 2x4KB + ps_proj 2x2KB = 16KB.
    with (
        tc.tile_pool(name="ps_sA", bufs=1, space="PSUM") as ps_sA,
        tc.tile_pool(name="ps_ht", bufs=2, space="PSUM") as ps_ht,
        tc.tile_pool(name="ps_proj", bufs=2, space="PSUM") as ps_proj,
    ):
        for c in range(NCH):
            htc = htp.tile([P, NT_E, CH], F16, name=f"htc{c}")
            for k in range(4):
                st = 4 * c + k
                ht_ps = ps_ht.tile([P, E], F32)
                # dependency-free keep-warm matmul: runs while the PE waits
                # for the H DMA, holding the HAM activity window busy so the
                # clock gate stays at K=8/8 through the DMA-paced phase A
                # (the transposes below overwrite the slot with start=True)
                nc.tensor.matmul(
                    ht_ps[:, 0:CH], warm[:, 0:P], warm[:], start=True, stop=True
                )
                nc.tensor.matmul(
                    ht_ps[:, 0:CH], warm[:, 0:P], warm[:], start=True, stop=True
                )
                for et in range(NT_E):
                    nc.tensor.matmul(
                        ht_ps[:, et * P : (et + 1) * P],
                        h_tiles[st][:, et * P : (et + 1) * P],
                        ident_h[:],
                        start=True,
                        stop=True,
                    )
                src = ht_ps.rearrange("p (t s) -> p t s", s=P)
                dst = htc[:, :, k * P : (k + 1) * P]
                # chunk 0 evacs ride ACT (idle until the first scores);
                # later chunks use the DVE so ACT stays exp-only.
                if c == 0:
                    nc.scalar.copy(dst, src)
                else:
                    nc.vector.tensor_copy(dst, src)
                for t in EARLY_SCHED.get((c, k), []):
                    emit_qkt_exp(ps_sA, *t)

            # qk projection
            qk_ps = ps_proj.tile([P, CH], F32, tag="pp")
            for et in range(NT_E):
                nc.tensor.matmul(
                    qk_ps[:],
                    wqk[:, et, :],
                    htc[:, et, :],
                    start=(et == 0),
                    stop=(et == NT_E - 1),
                )
            if c == 0:
                nc.scalar.activation(
                    qkT[:, c * CH : (c + 1) * CH], qk_ps[:], AF.Identity,
                    bias=bias_qk[:],
                )
            else:
                nc.vector.tensor_scalar_add(
                    qkT[:, c * CH : (c + 1) * CH], qk_ps[:], bias_qk[:]
                )
            # kT to partitions 0:64 / qT to partitions 64:128 (SBUF->SBUF
            # DMA on the HWDGE sync queue)
            nc.sync.dma_start(
                kT_lo[:, c * CH : (c + 1) * CH], qkT[D:P, c * CH : (c + 1) * CH]
            )
            nc.sync.dma_start(
                qT_hi[D:P, c * CH : (c + 1) * CH], qkT[0:D, c * CH : (c + 1) * CH]
            )

            # v projection
            vt_ps = ps_proj.tile([D, CH], F32, tag="pp")
            for et in range(NT_E):
                nc.tensor.matmul(
                    vt_ps[:],
                    wv_h[:, et, :],
                    htc[:, et, :],
                    start=(et == 0),
                    stop=(et == NT_E - 1),
                )
            if c == 0:
                nc.scalar.activation(
                    vT[:, c * CH : (c + 1) * CH], vt_ps[:], AF.Identity,
                    bias=bias_v[:],
                )
            else:
                nc.vector.tensor_scalar_add(
                    vT[:, c * CH : (c + 1) * CH], vt_ps[:], bias_v[:]
                )

            # transpose v for this chunk's 4 key tiles
            for jt in range(4 * c, 4 * c + 4):
                v_ps = ps_proj.tile([P, D], F32, tag="pp")
                nc.tensor.matmul(
                    v_ps[:],
                    vT[:, jt * P : (jt + 1) * P],
                    ident_h[0:D, 0:D],
                    start=True,
                    stop=True,
                )
                nc.vector.tensor_copy(v_sb[:, jt, 0:D], v_ps[:])

    # --- phase B: remaining attention + PV + epilogue ---------------------
    # PSUM: ps_s 2x4KB + ps_pv 2x4KB = 16KB.  Late QK^T+exp tiles are fed
    # one per PV tile consumed: emission stays ~8-12 tiles ahead of
    # consumption, so the in-order PE queue never blocks a ready PV on a
    # PSUM score slot, and ACT never starves.
    pending = deque(LATE)
    consumed_late = [0]
    with (
        tc.tile_pool(name="ps_s", bufs=3, space="PSUM") as ps_s,
        tc.tile_pool(name="ps_pv", bufs=1, space="PSUM") as ps_pv,
    ):
        def feed(force=False):
            # keep QK^T emission a bounded few tiles ahead of PV consumption
            # (sB is triple-buffered; a deeper lead head-blocks the PE queue)
            n_emitted = len(LATE) - len(pending)
            if pending and (force or n_emitted - consumed_late[0] < 4):
                emit_qkt_exp(ps_s, *pending.popleft())

        for _ in range(3):
            feed(force=True)
        for g in range(2):
            pv = ps_pv.tile([P, 2 * CH], F32, tag="pv")
            nmm = 0
            for p in PV_PSEQ:
                for ci in range(2):
                    feed()
                    c = 2 * g + ci
                    if (c, p) not in emitted:  # guard; pacing prevents this
                        pending.remove((c, p))
                        emit_qkt_exp(ps_s, c, p)
                    if (c, p) in LATE:
                        consumed_late[0] += 1
                # both c-halves per (p, dp) back to back: the v_sb stationary
                # is reloaded once instead of twice
                for dp in range(2):
                    for ci in range(2):
                        c = 2 * g + ci
                        nc.tensor.matmul(
                            pv[:, ci * CH : (ci + 1) * CH],
                            v_sb[:, 2 * p + dp, :],
                            expT[:, c * NPAIR + p, dp * CH : (dp + 1) * CH],
                            start=(nmm + ci < 2),
                            stop=(nmm + ci >= 30),
                        )
                    nmm += 2

            # epilogue for this 1024-query group (overlaps the feed of g=1's
            # remaining score tiles)
            pv_sb = outp.tile([D + 1, 2 * CH], F16, tag="pvsb")
            nc.vector.tensor_copy(pv_sb[:], pv[0 : D + 1, :])
            ot = ps_s.tile([P, 2 * CH], F32, tag="s")
            for k in range(8):
                if k % 2 == 0:
                    feed(force=True)
                nc.tensor.matmul(
                    ot[:, k * P : k * P + D + 1],
                    pv_sb[:, k * P : (k + 1) * P],
                    ident_h[0 : D + 1, 0 : D + 1],
                    start=True,
                    stop=True,
                )
            rcp = outp.tile([P, 8], F32, tag="rcp")
            ot3 = ot.rearrange("p (k c) -> p k c", c=P)
            nc.vector.reciprocal(
                rcp.rearrange("p (k one) -> p k one", one=1), ot3[:, :, D : D + 1]
            )
            # stores split in two pieces on the two DMA queues (sync HWDGE +
            # gpsimd SWDGE run concurrently) to shrink the end-of-kernel tail
            o_v = o.rearrange("(st p) d -> p st d", p=P)
            oa_v = o_acc.rearrange("p (st d) -> p st d", d=D)
            for k in range(8):
                st = 8 * g + k
                nc.vector.tensor_scalar_mul(
                    o_acc[:, st * D : (st + 1) * D], ot3[:, k, 0:D], rcp[:, k : k + 1]
                )
                if k % 2 == 1:  # store every 2 s-tiles, alternating queues
                    eng = nc.sync if (k // 2) % 2 == 0 else nc.gpsimd
                    eng.dma_start(
                        o_v[:, st - 1 : st + 1, :], oa_v[:, st - 1 : st + 1, :]
                    )


_NC_CACHE = None


def _build_nc():
    global _NC_CACHE
    if _NC_CACHE is not None:
        return _NC_CACHE
    nc = bacc.Bacc(
        "TRN2",
        target_bir_lowering=False,
        debug=False,
        enable_asserts=False,
        num_devices=B,
    )
    h = nc.dram_tensor("h", [S, E], F32, kind="ExternalInput").ap()
    wq_t = nc.dram_tensor("wq", [E, D], F32, kind="ExternalInput").ap()
    bq_t = nc.dram_tensor("bq", [D], F32, kind="ExternalInput").ap()
    wk_t = nc.dram_tensor("wk", [E, D], F32, kind="ExternalInput").ap()
    bk_t = nc.dram_tensor("bk", [D], F32, kind="ExternalInput").ap()
    wv_t = nc.dram_tensor("wv", [E, D], F32, kind="ExternalInput").ap()
    bv_t = nc.dram_tensor("bv", [D], F32, kind="ExternalInput").ap()
    o = nc.dram_tensor("o", [S, D], F32, kind="ExternalOutput").ap()
    with tile.TileContext(nc) as tc:
        with ExitStack() as ctx:
            _emit_kernel(ctx, tc, o, h, wq_t, bq_t, wk_t, bk_t, wv_t, bv_t)
    nc.compile()
    _NC_CACHE = nc
    return nc


def _run(inputs: dict, **kwargs):
    nc = _build_nc()
    f32c = lambda a: np.ascontiguousarray(np.asarray(a, dtype=np.float32))
    shared = {
        "wq": f32c(inputs["Wq"]),
        "bq": f32c(inputs["bq"]),
        "wk": f32c(inputs["Wk"]),
        "bk": f32c(inputs["bk"]),
        "wv": f32c(inputs["Wv"]),
        "bv": f32c(inputs["bv"]),
    }
    hs = f32c(inputs["hidden_state"])
    in_maps = [{"h": hs[b], **shared} for b in range(B)]
    res = run_bass_kernel_spmd(nc, in_maps, core_ids=list(range(B)), **kwargs)
    out = np.stack([res.results[b]["o"] for b in range(B)], axis=0)
    return out, res


def kernel(**inputs) -> np.ndarray:
    out, _ = _run(inputs)
    return out


# revision 17
# speedup vs baseline: 1.0606x; 1.0606x over previous
"""Single-head attention (B=8, S=2048, E=768, D=64) on 8 TRN2 NeuronCores.

Sharding: data-parallel over batch — one batch element per core; the small
Wq/Wk/Wv weights and biases are replicated to every core.

Per-core dataflow (fp16 matmul path, fp32 PSUM accumulation; rel err ~9e-4):

  Setup: exp-table load first on ACT; H s-tile loads stream on the SWDGE
  (gpsimd) queue with inline f32->fp16 cast (tile 0 split in 3 pieces so the
  PE can start sooner); the weight loads are SWDGE cast-loads slotted after
  the first 6 H tiles; the 1/sqrt(D) softmax scale is folded into the exp
  ACTIVATE's free affine (out = exp(scale*x)) so no weight prep is needed;
  8 fp16 warm-up matmuls keep the PE HAM activity window busy so the clock
  gate reaches K=8/8 before real work.

  Phase A (per 512-query chunk): PE-transpose H tiles against a fp16
  identity (normal matmuls, full-array activity), evacuate PSUM->SBUF on
  DVE (chunk 0 on ACT — it is otherwise idle until the first scores exist),
  project qkT = [Wq | Wk].T @ HT and vT = Wv.T @ HT with biases folded into
  the evacuation, copy kT down to partitions 0:64 (kT_lo) and qT up to
  partitions 64:128 (qT_hi) via SBUF->SBUF DMA on the HWDGE queue (which
  carries only the tiny bias loads besides), and PE-transpose v into
  per-key-tile [128,128] tiles (col 64 = 1.0: softmax-denominator trick).

  Attention: scores per (query-chunk c, key-pair p) tile are TWO CONCURRENT
  K=64 matmuls packed into PE row groups 0/64 (tile_position packing:
  kT_lo x qT in rows 0:64, qkT-hi x qT_hi in rows 64:128) -> one
  [128, 1024] PSUM tile; exp on ACT -> persistent SBUF fp16.  QK^T+exp for
  ready tiles are emitted INSIDE phase A (early attention) so ACT's serial
  exp stream — the largest single-engine cost, ~(1024+352)/1.2 ns x 32 —
  starts as soon as the first chunk is projected.  PV is deferred to
  phase B: per 1024-query group it accumulates v_sb.T @ expT over all 16
  key tiles (ones row = denominator), with the remaining QK^T+exp tiles
  fed into the stream at a rate of one per PV tile so the ACT pipeline
  never starves and PSUM score slots never block the in-order PE queue.

  Epilogue per query group: evacuate PV PSUM->SBUF as fp16 (DVE),
  PE-transpose back to [queries, 65], one strided reciprocal over all 8
  denominator columns, 8 tensor_scalar multiplies, store via HWDGE.

Softmax without max-subtraction is safe here: scores/8 ~ N(0,1) (max
|score/8| < ~8 over the whole problem), so exp() <= ~3000 and the fp16
denominators stay far below overflow; the result matches the
max-subtracted reference to fp32 rounding.
"""

from collections import deque
from contextlib import ExitStack

import numpy as np

import concourse.bacc as bacc
import concourse.mybir as mybir
import concourse.tile as tile
from concourse.bass_utils import run_bass_kernel_spmd

B = 8
S = 2048
E = 768
D = 64
P = 128
NT_S = S // P  # 16 key/s-tiles
NT_E = E // P  # 6 e-tiles
CH = 512  # query-chunk width
NCH = S // CH  # 4 query chunks
NPAIR = NT_S // 2  # 8 key-tile pairs
F32 = mybir.dt.float32
F16 = mybir.dt.float16
AF = mybir.ActivationFunctionType

SCALE = float(1.0 / np.sqrt(np.float32(D)))

# (chunk, pair) attention tiles emitted inside phase A, keyed by (chunk,
# s-tile-slot) DURING whose processing they are emitted.  Tile (c, p) needs
# qkT chunk c and kT tiles 2p,2p+1 (chunk p//2) plus the kT_lo/qT_hi copies
# of those chunks, so during chunk k anything with c <= k-1, p <= 2k-1 is
# ready (slot >= 1 leaves ~1 s-tile of slack for the copy DMA latency).
EARLY_SCHED = {
    (1, 1): [(0, 0)],
    (1, 2): [(0, 1)],
    (2, 1): [(1, 0)],
    (2, 2): [(1, 1)],
    (2, 3): [(0, 2), (0, 3)],
    (3, 0): [(1, 2)],
    (3, 1): [(1, 3), (2, 0)],
    (3, 2): [(2, 1), (2, 2)],
    (3, 3): [(2, 3)],
}
EARLY = [t for lst in EARLY_SCHED.values() for t in lst]
# Remaining tiles, in phase-B emission order, co-designed with the PV
# consumption order (late pairs p=4..7 first, early pairs p=0..3 last) so
# every group's PV stream ENDS on tiles whose exp finished long before.
LATE = [
    (0, 4), (1, 4), (0, 5), (1, 5), (0, 6), (1, 6), (0, 7), (1, 7),
    (2, 4), (3, 4), (2, 5), (3, 5), (2, 6), (3, 6), (2, 7), (3, 7),
    (3, 0), (3, 1), (3, 2), (3, 3),
]
# per-group PV consumption order over key pairs: start AND end on pairs
# whose exp is long done, with the fed-late pairs in the middle
PV_PSEQ = {0: [0, 1, 4, 5, 6, 7, 2, 3], 1: [4, 5, 6, 7, 0, 1, 2, 3]}


def _emit_kernel(ctx: ExitStack, tc: "tile.TileContext", o, h, wq, bq, wk, bk, wv, bv):
    nc = tc.nc

    const = ctx.enter_context(tc.tile_pool(name="const", bufs=1))
    hload = ctx.enter_context(tc.tile_pool(name="hload", bufs=1))
    htp = ctx.enter_context(tc.tile_pool(name="htp", bufs=2))
    big = ctx.enter_context(tc.tile_pool(name="bigsb", bufs=1))
    outp = ctx.enter_context(tc.tile_pool(name="outp", bufs=2))

    # --- setup ------------------------------------------------------------
    # Dummy exp first so the ACT exp table set loads during the DMA ramp.
    dummy = const.tile([1, 4], F32)
    nc.vector.memset(dummy[:], 0.0)
    nc.scalar.activation(dummy[:], dummy[:], AF.Exp)

    # H tile loads: SWDGE casts f32->fp16 inline.  Tile 0 is split in three
    # so its first third lands earlier and transposes start sooner.  The
    # weight cast-loads are slotted after the first 6 H tiles: early enough
    # for the first projection, without delaying h0.
    h_tiles = [hload.tile([P, E], F16, name=f"h{st}") for st in range(NT_S)]
    TE2 = 2 * P
    for piece in range(3):
        nc.gpsimd.dma_start(
            h_tiles[0][:, piece * TE2 : (piece + 1) * TE2],
            h[0:P, piece * TE2 : (piece + 1) * TE2],
        )
    nc.gpsimd.dma_start(h_tiles[1][:], h[P : 2 * P, :])

    # identity: DVE zeros + gpsimd diagonal (after h0/h1 descriptors so it
    # does not delay them; ready by the first transpose).
    ident = const.tile([P, P], F32)
    nc.vector.memset(ident[:], 0.0)
    nc.gpsimd.affine_select(
        out=ident[:],
        in_=ident[:],
        compare_op=mybir.AluOpType.not_equal,
        fill=1.0,
        base=0,
        pattern=[[-1, P]],
        channel_multiplier=1,
    )
    ident_h = const.tile([P, P], F16)
    nc.vector.tensor_copy(ident_h[:], ident[:])

    for st in range(2, NT_S):
        nc.gpsimd.dma_start(h_tiles[st][:], h[st * P : (st + 1) * P, :])

    # weights: f32 on the HWDGE sync queue (parallel to the H stream on the
    # SWDGE queue), cast to fp16 on the then-idle DVE
    wq_raw = const.tile([P, NT_E, D], F32)
    wk_raw = const.tile([P, NT_E, D], F32)
    wv_raw = const.tile([P, NT_E, D], F32)
    nc.sync.dma_start(wq_raw[:], wq.rearrange("(t p) d -> p t d", p=P))
    nc.sync.dma_start(wk_raw[:], wk.rearrange("(t p) d -> p t d", p=P))
    nc.sync.dma_start(wv_raw[:], wv.rearrange("(t p) d -> p t d", p=P))
    wqk = const.tile([P, NT_E, P], F16)  # cols 0:64 = Wq, 64:128 = Wk
    wv_h = const.tile([P, NT_E, D], F16)
    nc.vector.tensor_copy(wqk[:, :, 0:D], wq_raw[:])
    nc.vector.tensor_copy(wqk[:, :, D:P], wk_raw[:])
    nc.vector.tensor_copy(wv_h[:], wv_raw[:])

    # Warm-up: fp16 matmuls (~430ns each cold) keep the PE busy through the
    # HAM activity window while the first H tiles stream in.
    warm = const.tile([P, CH], F16)
    nc.vector.memset(warm[:], 1.0)
    with tc.tile_pool(name="ps_warm", bufs=1, space="PSUM") as ps_warm:
        warm_ps = ps_warm.tile([P, CH], F32)
        for _ in range(8):
            nc.tensor.matmul(warm_ps[:], warm[:, 0:P], warm[:], start=True, stop=True)

    # biases: tiny f32 loads on the otherwise-idle HWDGE sync queue
    bias_qk = const.tile([P, 1], F32)
    nc.sync.dma_start(bias_qk[0:D, :], bq.rearrange("(p one) -> p one", one=1))
    nc.sync.dma_start(bias_qk[D:P, :], bk.rearrange("(p one) -> p one", one=1))
    bias_v = const.tile([D, 1], F32)
    nc.sync.dma_start(bias_v[:], bv.rearrange("(p one) -> p one", one=1))

    # --- persistent SBUF --------------------------------------------------
    qkT = big.tile([P, S], F16)  # rows 0:64 = qT, rows 64:128 = kT
    qT_hi = big.tile([P, S], F16)  # rows 64:128 = qT (for row-group-64 QK^T)
    kT_lo = big.tile([D, S], F16)  # partitions 0:64 = kT (row-group-0 QK^T)
    vT = big.tile([D, S], F16)
    v_sb = big.tile([P, NT_S, P], F16)  # per key tile: [s, 0:64]=v, col64=1, rest 0
    nc.vector.memset(v_sb[:], 0.0)
    nc.vector.memset(v_sb[:, :, D : D + 1], 1.0)
    expT = big.tile([P, NCH * NPAIR, 2 * CH], F16)  # exp tile per (c, p)
    o_acc = big.tile([P, NT_S * D], F32)

    emitted = set()

    def emit_qkt_exp(pool, c, p):
        """Row-packed QK^T pair + exp (with the 1/8 scale folded in)."""
        s_ps = pool.tile([P, 2 * CH], F32, tag="s")
        nc.tensor.matmul(
            s_ps[:, 0:CH],
            kT_lo[:, 2 * p * P : (2 * p + 1) * P],
            qkT[0:D, c * CH : (c + 1) * CH],
            start=True,
            stop=True,
        )
        nc.tensor.matmul(
            s_ps[:, CH : 2 * CH],
            qkT[D:P, (2 * p + 1) * P : (2 * p + 2) * P],
            qT_hi[D:P, c * CH : (c + 1) * CH],
            start=True,
            stop=True,
        )
        nc.scalar.activation(expT[:, c * NPAIR + p, :], s_ps[:], AF.Exp, scale=SCALE)
        emitted.add((c, p))

    # --- phase A: transpose H, project, early attention -------------------
    # PSUM: ps_sA 1x4KB + ps_ht 2x4KB + ps_proj 2x2KB = 16KB.
    with (
        tc.tile_pool(name="ps_sA", bufs=1, space="PSUM") as ps_sA,
        tc.tile_pool(name="ps_ht", bufs=2, space="PSUM") as ps_ht,
        tc.tile_pool(name="ps_proj", bufs=2, space="PSUM") as ps_proj,
    ):
        for c in range(NCH):
            htc = htp.tile([P, NT_E, CH], F16, name=f"htc{c}")
            for k in range(4):
                st = 4 * c + k
                ht_ps = ps_ht.tile([P, E], F32)
                # dependency-free keep-warm matmul: runs while the PE waits
                # for the H DMA, holding the HAM activity window busy so the
                # clock gate stays at K=8/8 through the DMA-paced phase A
                # (the transposes below overwrite the slot with start=True)
                nc.tensor.matmul(
                    ht_ps[:, 0:CH], warm[:, 0:P], warm[:], start=True, stop=True
                )
                for et in range(NT_E):
                    nc.tensor.matmul(
                        ht_ps[:, et * P : (et + 1) * P],
                        h_tiles[st][:, et * P : (et + 1) * P],
                        ident_h[:],
                        start=True,
                        stop=True,
                    )
                src = ht_ps.rearrange("p (t s) -> p t s", s=P)
                dst = htc[:, :, k * P : (k + 1) * P]
                # chunk 0 evacs ride ACT (idle until the first scores);
                # later chunks use the DVE so ACT stays exp-only.
                if c == 0:
                    nc.scalar.copy(dst, src)
                else:
                    nc.vector.tensor_copy(dst, src)
                for t in EARLY_SCHED.get((c, k), []):
                    emit_qkt_exp(ps_sA, *t)

            # qk projection
            qk_ps = ps_proj.tile([P, CH], F32, tag="pp")
            for et in range(NT_E):
                nc.tensor.matmul(
                    qk_ps[:],
                    wqk[:, et, :],
                    htc[:, et, :],
                    start=(et == 0),
                    stop=(et == NT_E - 1),
                )
            if c == 0:
                nc.scalar.activation(
                    qkT[:, c * CH : (c + 1) * CH], qk_ps[:], AF.Identity,
                    bias=bias_qk[:],
                )
            else:
                nc.vector.tensor_scalar_add(
                    qkT[:, c * CH : (c + 1) * CH], qk_ps[:], bias_qk[:]
                )
            # kT to partitions 0:64 / qT to partitions 64:128 (SBUF->SBUF
            # DMA on the HWDGE sync queue)
            nc.sync.dma_start(
                kT_lo[:, c * CH : (c + 1) * CH], qkT[D:P, c * CH : (c + 1) * CH]
            )
            nc.sync.dma_start(
                qT_hi[D:P, c * CH : (c + 1) * CH], qkT[0:D, c * CH : (c + 1) * CH]
            )

            # v projection
            vt_ps = ps_proj.tile([D, CH], F32, tag="pp")
            for et in range(NT_E):
                nc.tensor.matmul(
                    vt_ps[:],
                    wv_h[:, et, :],
                    htc[:, et, :],
                    start=(et == 0),
                    stop=(et == NT_E - 1),
                )
            if c == 0:
                nc.scalar.activation(
                    vT[:, c * CH : (c + 1) * CH], vt_ps[:], AF.Identity,
                    bias=bias_v[:],
                )
            else:
                nc.vector.tensor_scalar_add(
                    vT[:, c * CH : (c + 1) * CH], vt_ps[:], bias_v[:]
                )

            # transpose v for this chunk's 4 key tiles
            for jt in range(4 * c, 4 * c + 4):
                v_ps = ps_proj.tile([P, D], F32, tag="pp")
                nc.tensor.matmul(
                    v_ps[:],
                    vT[:, jt * P : (jt + 1) * P],
                    ident_h[0:D, 0:D],
                    start=True,
                    stop=True,
                )
                nc.vector.tensor_copy(v_sb[:, jt, 0:D], v_ps[:])

    # --- phase B: remaining attention + PV + epilogue ---------------------
    # PSUM: ps_s 2x4KB + ps_pv 2x4KB = 16KB.  Late QK^T+exp tiles are fed
    # one per PV tile consumed: emission stays ~8-12 tiles ahead of
    # consumption, so the in-order PE queue never blocks a ready PV on a
    # PSUM score slot, and ACT never starves.
    pending = deque(LATE)
    with (
        tc.tile_pool(name="ps_s", bufs=2, space="PSUM") as ps_s,
        tc.tile_pool(name="ps_pv", bufs=2, space="PSUM") as ps_pv,
    ):
        def feed(force=False):
            if pending:
                emit_qkt_exp(ps_s, *pending.popleft())

        feed()
        for g in range(2):
            pv = ps_pv.tile([P, 2 * CH], F32, tag="pv")
            nmm = 0
            for p in PV_PSEQ[g]:
                for ci in range(2):
                    feed()
                    c = 2 * g + ci
                    if (c, p) not in emitted:  # guard; pacing prevents this
                        pending.remove((c, p))
                        emit_qkt_exp(ps_s, c, p)
                # both c-halves per (p, dp) back to back: the v_sb stationary
                # is reloaded once instead of twice
                for dp in range(2):
                    for ci in range(2):
                        c = 2 * g + ci
                        nc.tensor.matmul(
                            pv[:, ci * CH : (ci + 1) * CH],
                            v_sb[:, 2 * p + dp, :],
                            expT[:, c * NPAIR + p, dp * CH : (dp + 1) * CH],
                            start=(nmm + ci < 2),
                            stop=(nmm + ci >= 30),
                        )
                    nmm += 2

            # epilogue for this 1024-query group (overlaps the feed of g=1's
            # remaining score tiles)
            pv_sb = outp.tile([D + 1, 2 * CH], F16, tag="pvsb")
            nc.vector.tensor_copy(pv_sb[:], pv[0 : D + 1, :])
            ot = ps_s.tile([P, 2 * CH], F32, tag="s")
            for k in range(8):
                if k % 2 == 0:
                    feed(force=True)
                nc.tensor.matmul(
                    ot[:, k * P : k * P + D + 1],
                    pv_sb[:, k * P : (k + 1) * P],
                    ident_h[0 : D + 1, 0 : D + 1],
                    start=True,
                    stop=True,
                )
            rcp = outp.tile([P, 8], F32, tag="rcp")
            ot3 = ot.rearrange("p (k c) -> p k c", c=P)
            nc.vector.reciprocal(
                rcp.rearrange("p (k one) -> p k one", one=1), ot3[:, :, D : D + 1]
            )
            # stores split in two pieces on the two DMA queues (sync HWDGE +
            # gpsimd SWDGE run concurrently) to shrink the end-of-kernel tail
            o_v = o.rearrange("(st p) d -> p st d", p=P)
            oa_v = o_acc.rearrange("p (st d) -> p st d", d=D)
            for k in range(8):
                st = 8 * g + k
                nc.vector.tensor_scalar_mul(
                    o_acc[:, st * D : (st + 1) * D], ot3[:, k, 0:D], rcp[:, k : k + 1]
                )
                if k % 2 == 1:  # store every 2 s-tiles, alternating queues
                    eng = nc.sync if (k // 2) % 2 == 0 else nc.gpsimd
                    eng.dma_start(
                        o_v[:, st - 1 : st + 1, :], oa_v[:, st - 1 : st + 1, :]
                    )


_NC_CACHE = None


def _build_nc():
    global _NC_CACHE
    if _NC_CACHE is not None:
        return _NC_CACHE
    nc = bacc.Bacc(
        "TRN2",
        target_bir_lowering=False,
        debug=False,
        enable_asserts=False,
        num_devices=B,
    )
    h = nc.dram_tensor("h", [S, E], F32, kind="ExternalInput").ap()
    wq_t = nc.dram_tensor("wq", [E, D], F32, kind="ExternalInput").ap()
    bq_t = nc.dram_tensor("bq", [D], F32, kind="ExternalInput").ap()
    wk_t = nc.dram_tensor("wk", [E, D], F32, kind="ExternalInput").ap()
    bk_t = nc.dram_tensor("bk", [D], F32, kind="ExternalInput").ap()
    wv_t = nc.dram_tensor("wv", [E, D], F32, kind="ExternalInput").ap()
    bv_t = nc.dram_tensor("bv", [D], F32, kind="ExternalInput").ap()
    o = nc.dram_tensor("o", [S, D], F32, kind="ExternalOutput").ap()
    with tile.TileContext(nc) as tc:
        with ExitStack() as ctx:
            _emit_kernel(ctx, tc, o, h, wq_t, bq_t, wk_t, bk_t, wv_t, bv_t)
    nc.compile()
    _NC_CACHE = nc
    return nc


def _run(inputs: dict, **kwargs):
    nc = _build_nc()
    f32c = lambda a: np.ascontiguousarray(np.asarray(a, dtype=np.float32))
    shared = {
        "wq": f32c(inputs["Wq"]),
        "bq": f32c(inputs["bq"]),
        "wk": f32c(inputs["Wk"]),
        "bk": f32c(inputs["bk"]),
        "wv": f32c(inputs["Wv"]),
        "bv": f32c(inputs["bv"]),
    }
    hs = f32c(inputs["hidden_state"])
    in_maps = [{"h": hs[b], **shared} for b in range(B)]
    res = run_bass_kernel_spmd(nc, in_maps, core_ids=list(range(B)), **kwargs)
    out = np.stack([res.results[b]["o"] for b in range(B)], axis=0)
    return out, res


def kernel(**inputs) -> np.ndarray:
    out, _ = _run(inputs)
    return out
